# revision 1
# baseline (speedup 1.0000x reference)
"""3x3 valid conv via 1D Winograd F(2,3) along H, on 8 Trainium2 cores.

x: (16, 128, 64, 64) f32, weights: (256, 128, 3, 3) f32
-> out: (16, 256, 62, 62) f32

Data-parallel, 2 images per core. Output rows come in pairs (31 tiles):
  V0 = x[2t]   - x[2t+2]        M[v] = sum_kj Wt[v,kj]^T V[v][.., kj:kj+62]
  V1 = x[2t+1] + x[2t+2]        y[2t]   = M0 + M1 + M2
  V2 = x[2t+2] - x[2t+1]        y[2t+1] = M1 - M2 - M3
  V3 = x[2t+1] - x[2t+3]
12 matmuls per 16-row output block instead of 18 -- 1.5x less PE work.
The V transform (pure shifted adds) and the G weight transform run on
the host; the device streams V/Wt in bf16 (half the DMA of f32 x) and
does matmuls plus the output combine.

Per block: 4 PSUM banks M0..M3; Act copies M1 to SBUF (c1), then DVE
does te=c1+M0, y_even=te+M2, to=c1-M2, y_odd=to-M3 (each reads at most
one PSUM operand -- hw limit). GpSimd is kept idle: its DSP ops are
slow (~1.2us per 496-elem op) and its activity tightens the HAM
throttle window. Stores go out on the Act HWDGE ring.
"""

import numpy as np

N_CORES = 8
IMGS_PER_CORE = 2
CIN = 128
COUT = 256
H = W = 64
OH = OW = 62
NT = 31  # output row-tiles per image
TBS = [(0, 8), (8, 8), (16, 8), (24, 7)]  # (t0, ntiles) blocks

_NC_CACHE = []


def _build():
    import concourse.bacc as bacc
    import concourse.mybir as mybir
    import concourse.tile as tile

    bf16 = mybir.dt.bfloat16
    f32 = mybir.dt.float32

    nc = bacc.Bacc("TRN2", target_bir_lowering=False, debug=False)
    # V layout: [img, cin, v, t, col]
    v_in = nc.dram_tensor(
        "v", [IMGS_PER_CORE, CIN, 4, NT, W], bf16, kind="ExternalInput"
    ).ap()
    # w layout: [cin, (h, v, kj, coutl)]
    w = nc.dram_tensor("w", [CIN, 2 * 4 * 3 * 128], bf16, kind="ExternalInput").ap()
    out = nc.dram_tensor(
        "out", [IMGS_PER_CORE, COUT, OH, OW], f32, kind="ExternalOutput"
    ).ap()

    with tile.TileContext(nc) as tc:
        with (
            tc.tile_pool(name="wp", bufs=1) as w_pool,
            tc.tile_pool(name="vp", bufs=1) as v_pool,
            tc.tile_pool(name="yp", bufs=4) as y_pool,
            tc.tile_pool(name="tep", bufs=4) as te_pool,
            tc.tile_pool(name="cp", bufs=4) as c_pool,
            tc.tile_pool(name="ps", bufs=8, space="PSUM") as ps_pool,
        ):
            # PE prewarm through the initial DMA window. memset on DVE, NOT
            # gpsimd: the gpsimd sequencer boots ~6us late and would gate
            # the prewarm; gpsimd activity also tightens the HAM throttle.
            scr = w_pool.tile([CIN, 128], f32, tag="scr")
            nc.vector.memset(scr[:], 0.0)
            wrm = ps_pool.tile([128, 8, OW], f32, name="wrm", tag="p")
            for _ in range(7):
                nc.tensor.matmul(
                    wrm[:, :2, :], scr[:], scr[:, :124], start=True, stop=True
                )

            w_sb = w_pool.tile([CIN, 2 * 4 * 3 * 128], bf16, tag="w")
            vts = {
                0: v_pool.tile([CIN, 4, NT, W], bf16, name="v0", tag="v0"),
                1: v_pool.tile([CIN, 4, NT, W], bf16, name="v1", tag="v1"),
            }
            # two rings in parallel for the startup-critical transfers: sync
            # takes the h0 weights while scalar takes the first V block, so
            # the first real matmul waits on neither serially. The rest of V
            # streams on sync in consumption order; h1 weights on scalar.
            # h0 weights arrive in nu-consumption order (1,2 then 0 then 3)
            nc.sync.dma_start(w_sb[:, 384:1152], w[:, 384:1152])  # h0 nu1,nu2
            nc.scalar.dma_start(vts[0][:, :, 0:8, :], v_in[0, :, :, 0:8, :])
            nc.sync.dma_start(w_sb[:, :384], w[:, :384])  # h0 nu0
            nc.sync.dma_start(w_sb[:, 1152:1536], w[:, 1152:1536])  # h0 nu3
            nc.scalar.dma_start(w_sb[:, 1536:], w[:, 1536:])  # h1
            for img in range(IMGS_PER_CORE):
                for t0, T in TBS:
                    if img == 0 and t0 == 0:
                        continue
                    nc.sync.dma_start(
                        vts[img][:, :, t0 : t0 + T, :],
                        v_in[img, :, :, t0 : t0 + T, :],
                    )

            def do_group(img, h, tb, split=1, sub=None):
                t0, T = TBS[tb]
                if sub is not None:
                    t0, T = sub
                v = vts[img]
                P = [None] * 4
                # nu order 1,2,0,3: M1/M2 finish early so the Act copy and
                # the c1-P2 combine start sooner, freeing their banks earlier
                for nu in (1, 2, 0, 3):
                    p = ps_pool.tile([128, 8, OW], f32, name="p", tag="p")
                    P[nu] = p
                    for kj in range(3):
                        wsl = w_sb[:, (h * 4 + nu) * 384 + kj * 128:
                                   (h * 4 + nu) * 384 + kj * 128 + 128]
                        nc.tensor.matmul(
                            p[:, :T, :],
                            wsl,
                            v[:, nu, t0 : t0 + T, kj : kj + OW],
                            start=(kj == 0),
                            stop=(kj == 2),
                        )
                # y[2t] = M0+M1+M2 ; y[2t+1] = M1-M2-M3
                # (split>1 halves the combine/store granularity so the final
                # block drains faster after the last matmul)
                c1 = c_pool.tile([128, 8, OW], f32, name="c1")
                nc.scalar.copy(c1[:, :T, :], P[1][:, :T, :])
                te = te_pool.tile([128, 8, OW], f32, name="te")
                to = te_pool.tile([128, 8, OW], f32, name="to")
                y = y_pool.tile([128, 16, OW], f32, name="y")
                bounds = [(0, T)] if split == 1 else [(0, T // 2), (T // 2, T)]
                for a, b in bounds:
                    nc.vector.tensor_add(te[:, a:b, :], c1[:, a:b, :], P[0][:, a:b, :])
                    nc.vector.tensor_add(
                        y[:, 2 * a : 2 * b : 2, :], te[:, a:b, :], P[2][:, a:b, :]
                    )
                    nc.vector.tensor_sub(to[:, a:b, :], c1[:, a:b, :], P[2][:, a:b, :])
                    nc.vector.tensor_sub(
                        y[:, 2 * a + 1 : 2 * b : 2, :], to[:, a:b, :], P[3][:, a:b, :]
                    )
                    nc.scalar.dma_start(
                        out[img, h * 128 : h * 128 + 128,
                            2 * (t0 + a) : 2 * (t0 + b), :],
                        y[:, 2 * a : 2 * b, :],
                    )

            # tb-major, h-inner: each V block feeds two consecutive groups,
            # halving the input-DMA rate the PE rampup needs. The very last
            # block runs as two small sub-groups (T=4 then T=3) so the
            # post-last-matmul combine/store chain is short.
            for img in range(IMGS_PER_CORE):
                for tb in range(4):
                    for h in range(2):
                        last = img == IMGS_PER_CORE - 1 and tb == 3 and h == 1
                        if last:
                            do_group(img, h, tb, sub=(24, 4))
                            do_group(img, h, tb, sub=(28, 3))
                        else:
                            do_group(img, h, tb)
    nc.compile()
    return nc


def _get_nc():
    if not _NC_CACHE:
        _NC_CACHE.append(_build())
    return _NC_CACHE[0]


def _pack_weights(weights):
    # [cout, cin, kh, kw] -> Wt[cin, h, v, kj, coutl] -> [cin, 2*4*3*128]
    import ml_dtypes

    G = np.array(
        [[1, 0, 0], [0.5, 0.5, 0.5], [0.5, -0.5, 0.5], [0, 0, 1]], dtype=np.float64
    )
    wt = weights.astype(np.float64).reshape(2, 128, CIN, 3, 3)  # h,o,i,ki,kj
    wtr = np.einsum("vk,hoikj->ihvjo", G, wt)  # i,h,v,kj,o
    return np.ascontiguousarray(wtr.reshape(CIN, 2 * 4 * 3 * 128)).astype(
        ml_dtypes.bfloat16
    )


def _pack_v(x):
    # x [n, cin, 64, 64] f32 -> V [n, cin, 4, 31, 64] bf16
    import ml_dtypes

    a0 = x[:, :, 0:62:2, :]
    a1 = x[:, :, 1:63:2, :]
    a2 = x[:, :, 2:64:2, :]
    a3 = x[:, :, 3:64:2, :]
    v = np.stack((a0 - a2, a1 + a2, a2 - a1, a1 - a3), axis=2)
    return np.ascontiguousarray(v).astype(ml_dtypes.bfloat16)


def _make_in_maps(x, weights):
    xv = _pack_v(np.ascontiguousarray(x, dtype=np.float32))
    w_l = _pack_weights(np.ascontiguousarray(weights, dtype=np.float32))
    return [
        {"v": xv[IMGS_PER_CORE * c : IMGS_PER_CORE * (c + 1)], "w": w_l}
        for c in range(N_CORES)
    ]


def _ldw_opt_patch():
    """No-op (kept for harness compat)."""
    import contextlib

    return contextlib.nullcontext()


def kernel(x, weights):
    from concourse.bass_utils import run_bass_kernel_spmd

    nc = _get_nc()
    in_maps = _make_in_maps(x, weights)
    res = run_bass_kernel_spmd(nc, in_maps, core_ids=list(range(N_CORES)))
    return np.concatenate([r["out"] for r in res.results], axis=0)



# revision 2
# speedup vs baseline: 1.0215x; 1.0215x over previous
"""3x3 valid conv via 1D Winograd F(4,3) along H, on 8 Trainium2 cores,
with the output (A^T) combine moved to the HOST.

x: (16, 128, 64, 64) f32, weights: (256, 128, 3, 3) f32
-> out: (16, 256, 62, 62) f32

Data-parallel, 2 images per core. Interpolation points {0, 1, -1, 1/2, -2, inf}
(chosen to minimize bf16 transform error; classic {0,±1,±2} overflows the
2e-2 gate). 16 row-tiles of 4 output rows (tile 15 overlaps: rows 58-61).

Per tile: 6 nu x 3 kj = 18 matmuls per 4 output rows (vs 12 per 2 rows for
F(2,3)) -> 25% less PE work, and the device does NO output combine at all:
each PSUM bank M[nu] is copied to SBUF as bf16 (DVE/Act alternating) and
DMA'd out; the host applies y = A^T M in f32. This removes the DVE
tensor_tensor wall (the F(2,3) kernel's combine trailed the PE stream) and
cuts store bytes to 6.1MB bf16.

V transform (B^T, host, f64->bf16) and G weight transform also on host.
GpSimd stays idle (slow DSP ops; tightens the HAM throttle window).
Loads ride the sync HWDGE ring, stores + early V chunks on the Act ring.
"""

import numpy as np

N_CORES = 8
IMGS_PER_CORE = 2
CIN = 128
COUT = 256
H = W = 64
OH = OW = 62
NT = 16            # row-tiles per image (4 output rows each; tile 15 at row 58)
NNU = 6
TBS = [(0, 8), (8, 8)]  # (t0, ntiles) blocks; one PSUM bank per nu holds 8 tiles

# F(4,3) with points {0, 1, -1, 1/2, -2, inf}
_AT = np.array(
    [
        [1.0, 1.0, 1.0, 1.0, 1.0, 0.0],
        [0.0, 1.0, -1.0, 0.5, -2.0, 0.0],
        [0.0, 1.0, 1.0, 0.25, 4.0, 0.0],
        [0.0, 1.0, -1.0, 0.125, -8.0, 1.0],
    ]
)
_G = np.array(
    [
        [1.0, 0.0, 0.0],
        [1 / 3, 1 / 3, 1 / 3],
        [-1 / 3, 1 / 3, -1 / 3],
        [-16 / 15, -8 / 15, -4 / 15],
        [1 / 15, -2 / 15, 4 / 15],
        [0.0, 0.0, 1.0],
    ]
)
_BT = np.array(
    [
        [1.0, -1.5, -2.0, 1.5, 1.0, 0.0],
        [0.0, -1.0, 0.5, 2.5, 1.0, 0.0],
        [0.0, 1.0, -2.5, 0.5, 1.0, 0.0],
        [0.0, -2.0, -1.0, 2.0, 1.0, 0.0],
        [0.0, 0.5, -1.0, -0.5, 1.0, 0.0],
        [0.0, 1.0, -1.5, -2.0, 1.5, 1.0],
    ]
)
_STARTS = np.array([0, 4, 8, 12, 16, 20, 24, 28, 32, 36, 40, 44, 48, 52, 56, 58])

_NC_CACHE = []


def _build():
    import concourse.bacc as bacc
    import concourse.mybir as mybir
    import concourse.tile as tile

    bf16 = mybir.dt.bfloat16
    f32 = mybir.dt.float32

    nc = bacc.Bacc("TRN2", target_bir_lowering=False, debug=False)
    # V layout: [img, cin, t, nu, col] (t-major so one tb block is contiguous)
    v_in = nc.dram_tensor(
        "v", [IMGS_PER_CORE, CIN, NT, NNU, W], bf16, kind="ExternalInput"
    ).ap()
    # w layout: [cin, (h, nu, kj, coutl)]
    w = nc.dram_tensor(
        "w", [CIN, 2 * NNU * 3 * 128], bf16, kind="ExternalInput"
    ).ap()
    # M out: [img, h, coutl, nu, t, col]
    m_out = nc.dram_tensor(
        "m", [IMGS_PER_CORE, 2, 128, NNU, NT, OW], bf16, kind="ExternalOutput"
    ).ap()

    with tile.TileContext(nc) as tc:
        with (
            tc.tile_pool(name="wp", bufs=1) as w_pool,
            tc.tile_pool(name="vp", bufs=1) as v_pool,
            tc.tile_pool(name="sp", bufs=4) as s_pool,
            tc.tile_pool(name="ps", bufs=8, space="PSUM") as ps_pool,
        ):
            # PE prewarm through the initial DMA window. memset on DVE, NOT
            # gpsimd (gpsimd boots late and tightens the HAM throttle).
            scr = w_pool.tile([CIN, 128], f32, tag="scr")
            nc.vector.memset(scr[:], 0.0)
            wrm = ps_pool.tile([128, 8, OW], f32, name="wrm", tag="p")
            for _ in range(7):
                nc.tensor.matmul(
                    wrm[:, :2, :], scr[:], scr[:, :124], start=True, stop=True
                )

            w_sb = w_pool.tile([CIN, 2 * NNU * 3 * 128], bf16, tag="w")
            vts = {
                0: v_pool.tile([CIN, NT, NNU, W], bf16, name="v0", tag="v0"),
                1: v_pool.tile([CIN, NT, NNU, W], bf16, name="v1", tag="v1"),
            }
            # Startup choreography: two rings in parallel. First group needs
            # w[h0,nu01] + v[img0,tb0,nu01] -> both land first on their rings;
            # later chunks arrive in consumption order.
            nc.sync.dma_start(w_sb[:, 0:768], w[:, 0:768])  # h0 nu0,1
            nc.scalar.dma_start(vts[0][:, 0:8, 0:2, :], v_in[0, :, 0:8, 0:2, :])
            nc.sync.dma_start(w_sb[:, 768:2304], w[:, 768:2304])  # h0 nu2-5
            nc.scalar.dma_start(vts[0][:, 0:8, 2:4, :], v_in[0, :, 0:8, 2:4, :])
            nc.sync.dma_start(w_sb[:, 2304:4608], w[:, 2304:4608])  # h1
            nc.scalar.dma_start(vts[0][:, 0:8, 4:6, :], v_in[0, :, 0:8, 4:6, :])
            nc.sync.dma_start(vts[0][:, 8:16, :, :], v_in[0, :, 8:16, :, :])
            nc.sync.dma_start(vts[1][:, 0:8, :, :], v_in[1, :, 0:8, :, :])
            nc.sync.dma_start(vts[1][:, 8:16, :, :], v_in[1, :, 8:16, :, :])

            def do_group(img, h, tb, sub=None):
                t0, T = TBS[tb]
                if sub is not None:
                    t0, T = sub
                v = vts[img]
                stg = s_pool.tile([128, NNU, 8, OW], bf16, name="stg")
                for nu in range(NNU):
                    p = ps_pool.tile([128, 8, OW], f32, name="p", tag="p")
                    for kj in range(3):
                        wsl = w_sb[
                            :,
                            ((h * NNU + nu) * 3 + kj) * 128 :
                            ((h * NNU + nu) * 3 + kj) * 128 + 128,
                        ]
                        nc.tensor.matmul(
                            p[:, :T, :],
                            wsl,
                            v[:, t0 : t0 + T, nu, kj : kj + OW],
                            start=(kj == 0),
                            stop=(kj == 2),
                        )
                    # PSUM -> SBUF bf16; alternate engines so each keeps pace
                    # with the 3-MM step of the PE stream
                    if nu % 2 == 0:
                        nc.vector.tensor_copy(stg[:, nu, :T, :], p[:, :T, :])
                    else:
                        nc.scalar.copy(stg[:, nu, :T, :], p[:, :T, :])
                nc.scalar.dma_start(
                    m_out[img, h, :, :, t0 : t0 + T, :], stg[:, :, :T, :]
                )

            # tb-major, h-inner: each V block feeds two consecutive groups.
            # The very last block runs as two T=4 sub-groups so the
            # post-last-matmul copy/store chain is short.
            for img in range(IMGS_PER_CORE):
                for tb in range(2):
                    for h in range(2):
                        last = img == IMGS_PER_CORE - 1 and tb == 1 and h == 1
                        if last:
                            do_group(img, h, tb, sub=(8, 4))
                            do_group(img, h, tb, sub=(12, 4))
                        else:
                            do_group(img, h, tb)
    nc.compile()
    return nc


def _get_nc():
    if not _NC_CACHE:
        _NC_CACHE.append(_build())
    return _NC_CACHE[0]


def _pack_weights(weights):
    # [cout, cin, kh, kw] -> Wt[v,kj,o,c] -> [cin, (h, nu, kj, coutl)]
    import ml_dtypes

    wt = np.einsum("vk,ockj->vjoc", _G, weights.astype(np.float64))
    # reorder to [c, h, nu, kj, ol]
    wt = wt.reshape(NNU, 3, 2, 128, CIN)  # v, kj, h, ol, c
    wtr = np.transpose(wt, (4, 2, 0, 1, 3))  # c, h, v, kj, ol
    return np.ascontiguousarray(wtr.reshape(CIN, 2 * NNU * 3 * 128)).astype(
        ml_dtypes.bfloat16
    )


def _pack_v(x):
    # x [n, cin, 64, 64] f32 -> V [n, cin, 16, 6, 64] bf16
    import ml_dtypes

    idx = _STARTS[:, None] + np.arange(6)[None, :]  # (16, 6)
    xg = x[:, :, idx, :]  # (n, c, 16, 6, 64)
    v = np.einsum("vj,nctjw->nctvw", _BT.astype(np.float32), xg)
    return np.ascontiguousarray(v).astype(ml_dtypes.bfloat16)


def _make_in_maps(x, weights):
    xv = _pack_v(np.ascontiguousarray(x, dtype=np.float32))
    w_l = _pack_weights(np.ascontiguousarray(weights, dtype=np.float32))
    return [
        {"v": xv[IMGS_PER_CORE * c : IMGS_PER_CORE * (c + 1)], "w": w_l}
        for c in range(N_CORES)
    ]


def _ldw_opt_patch():
    """No-op (kept for harness compat)."""
    import contextlib

    return contextlib.nullcontext()


def _combine(m_all):
    # m_all: [16, 2, 128, 6, 16, 62] bf16 -> out [16, 256, 62, 62] f32
    m = np.asarray(m_all, dtype=np.float32)
    n = m.shape[0]
    m = m.reshape(n, 256, NNU, NT, OW)  # couts = h*128 + ol
    y = np.einsum("rv,novtu->notru", _AT.astype(np.float32), m)  # n,o,t,r,u
    out = np.empty((n, COUT, OH, OW), dtype=np.float32)
    for t in range(NT):
        out[:, :, _STARTS[t] : _STARTS[t] + 4, :] = y[:, :, t, :, :]
    return out


def kernel(x, weights):
    from concourse.bass_utils import run_bass_kernel_spmd

    nc = _get_nc()
    in_maps = _make_in_maps(x, weights)
    res = run_bass_kernel_spmd(nc, in_maps, core_ids=list(range(N_CORES)))
    m_all = np.concatenate([r["m"] for r in res.results], axis=0)
    return _combine(m_all)


# revision 6
# speedup vs baseline: 1.1650x; 1.1404x over previous
"""3x3 valid conv via 1D Winograd F(4,3) along H, on 8 Trainium2 cores,
with the output (A^T) combine moved to the HOST.

x: (16, 128, 64, 64) f32, weights: (256, 128, 3, 3) f32
-> out: (16, 256, 62, 62) f32

Data-parallel, 2 images per core. Interpolation points {0, 1, -1, 1/2, -2, inf}
(chosen to minimize bf16 transform error; classic {0,±1,±2} overflows the
2e-2 gate). 16 row-tiles of 4 output rows (tile 15 overlaps: rows 58-61).

Per tile: 6 nu x 3 kj = 18 matmuls per 4 output rows (vs 12 per 2 rows for
F(2,3)) -> 25% less PE work, and the device does NO output combine at all:
each PSUM bank M[nu] is copied to SBUF as bf16 (DVE/Act alternating) and
DMA'd out; the host applies y = A^T M in f32. This removes the DVE
tensor_tensor wall (the F(2,3) kernel's combine trailed the PE stream) and
cuts store bytes to 6.1MB bf16.

V transform (B^T, host, f64->bf16) and G weight transform also on host.
GpSimd stays idle (slow DSP ops; tightens the HAM throttle window).
Loads ride the sync HWDGE ring, stores + early V chunks on the Act ring.
"""

import numpy as np

N_CORES = 8
IMGS_PER_CORE = 2
CIN = 128
COUT = 256
H = W = 64
OH = OW = 62
NT = 16            # row-tiles per image (4 output rows each; tile 15 at row 58)
NNU = 6
TBS = [(0, 8), (8, 8)]  # (t0, ntiles) blocks; one PSUM bank per nu holds 8 tiles

# F(4,3) with points {0, 1, -1, 1/2, -2, inf}
_AT = np.array(
    [
        [1.0, 1.0, 1.0, 1.0, 1.0, 0.0],
        [0.0, 1.0, -1.0, 0.5, -2.0, 0.0],
        [0.0, 1.0, 1.0, 0.25, 4.0, 0.0],
        [0.0, 1.0, -1.0, 0.125, -8.0, 1.0],
    ]
)
_G = np.array(
    [
        [1.0, 0.0, 0.0],
        [1 / 3, 1 / 3, 1 / 3],
        [-1 / 3, 1 / 3, -1 / 3],
        [-16 / 15, -8 / 15, -4 / 15],
        [1 / 15, -2 / 15, 4 / 15],
        [0.0, 0.0, 1.0],
    ]
)
_BT = np.array(
    [
        [1.0, -1.5, -2.0, 1.5, 1.0, 0.0],
        [0.0, -1.0, 0.5, 2.5, 1.0, 0.0],
        [0.0, 1.0, -2.5, 0.5, 1.0, 0.0],
        [0.0, -2.0, -1.0, 2.0, 1.0, 0.0],
        [0.0, 0.5, -1.0, -0.5, 1.0, 0.0],
        [0.0, 1.0, -1.5, -2.0, 1.5, 1.0],
    ]
)
_STARTS = np.array([0, 4, 8, 12, 16, 20, 24, 28, 32, 36, 40, 44, 48, 52, 56, 58])

_NC_CACHE = []


def _build():
    import concourse.bacc as bacc
    import concourse.mybir as mybir
    import concourse.tile as tile

    bf16 = mybir.dt.bfloat16
    f32 = mybir.dt.float32

    nc = bacc.Bacc("TRN2", target_bir_lowering=False, debug=False)
    # V layout: [img, cin, nu, t, col] (nu-major: per-nu chunks are 1024B
    # contiguous per partition -- 256B elements run ~10x slower on the rings)
    v_in = nc.dram_tensor(
        "v", [IMGS_PER_CORE, CIN, NNU, NT, W], bf16, kind="ExternalInput"
    ).ap()
    # w layout: [cin, (h, nu, kj, coutl)]
    w = nc.dram_tensor(
        "w", [CIN, 2 * NNU * 3 * 128], bf16, kind="ExternalInput"
    ).ap()
    # M out: [img, h, coutl, nu, t, col]
    m_out = nc.dram_tensor(
        "m", [IMGS_PER_CORE, 2, 128, NNU, NT, OW], bf16, kind="ExternalOutput"
    ).ap()

    with tile.TileContext(nc) as tc:
        with (
            tc.tile_pool(name="wp", bufs=1) as w_pool,
            tc.tile_pool(name="vp", bufs=1) as v_pool,
            tc.tile_pool(name="sp", bufs=4) as s_pool,
            tc.tile_pool(name="ps", bufs=8, space="PSUM") as ps_pool,
        ):
            # PE prewarm through the initial DMA window. memset on DVE, NOT
            # gpsimd (gpsimd boots late and tightens the HAM throttle).
            scr = w_pool.tile([CIN, 128], f32, tag="scr")
            nc.vector.memset(scr[:], 0.0)
            wrm = ps_pool.tile([128, 8, OW], f32, name="wrm", tag="p")
            for _ in range(6):
                nc.tensor.matmul(
                    wrm[:, :2, :], scr[:], scr[:, :124], start=True, stop=True
                )

            w_sb = w_pool.tile([CIN, 2 * NNU * 3 * 128], bf16, tag="w")
            vts = {
                0: v_pool.tile([CIN, NNU, NT, W], bf16, name="v0", tag="v0"),
                1: v_pool.tile([CIN, NNU, NT, W], bf16, name="v1", tag="v1"),
            }
            # Startup choreography: two rings in parallel. First group needs
            # w[h0,nu01] + v[img0,tb0,nu01] -> both land first on their rings;
            # later chunks arrive in consumption order.
            nc.sync.dma_start(w_sb[:, 0:768], w[:, 0:768])  # h0 nu0,1
            nc.scalar.dma_start(vts[0][:, 0:2, 0:8, :], v_in[0, :, 0:2, 0:8, :])
            nc.sync.dma_start(w_sb[:, 768:2304], w[:, 768:2304])  # h0 nu2-5
            nc.scalar.dma_start(vts[0][:, 2:4, 0:8, :], v_in[0, :, 2:4, 0:8, :])
            nc.sync.dma_start(w_sb[:, 2304:4608], w[:, 2304:4608])  # h1
            nc.scalar.dma_start(vts[0][:, 4:6, 0:8, :], v_in[0, :, 4:6, 0:8, :])
            nc.sync.dma_start(vts[0][:, :, 8:16, :], v_in[0, :, :, 8:16, :])
            nc.sync.dma_start(vts[1][:, :, 0:8, :], v_in[1, :, :, 0:8, :])
            nc.sync.dma_start(vts[1][:, :, 8:16, :], v_in[1, :, :, 8:16, :])

            def do_group(img, h, tb, sub=None):
                t0, T = TBS[tb]
                if sub is not None:
                    t0, T = sub
                v = vts[img]
                stg = s_pool.tile([128, NNU, 8, OW], bf16, name="stg")
                for nu in range(NNU):
                    p = ps_pool.tile([128, 8, OW], f32, name="p", tag="p")
                    for kj in range(3):
                        wsl = w_sb[
                            :,
                            ((h * NNU + nu) * 3 + kj) * 128 :
                            ((h * NNU + nu) * 3 + kj) * 128 + 128,
                        ]
                        nc.tensor.matmul(
                            p[:, :T, :],
                            wsl,
                            v[:, nu, t0 : t0 + T, kj : kj + OW],
                            start=(kj == 0),
                            stop=(kj == 2),
                        )
                    # PSUM -> SBUF bf16; alternate engines so each keeps pace
                    # with the 3-MM step of the PE stream
                    if nu % 2 == 0:
                        nc.vector.tensor_copy(stg[:, nu, :T, :], p[:, :T, :])
                    else:
                        nc.scalar.copy(stg[:, nu, :T, :], p[:, :T, :])
                nc.scalar.dma_start(
                    m_out[img, h, :, :, t0 : t0 + T, :], stg[:, :, :T, :]
                )

            # tb-major, h-inner: each V block feeds two consecutive groups.
            # The very last block runs as two T=4 sub-groups so the
            # post-last-matmul copy/store chain is short.
            for img in range(IMGS_PER_CORE):
                for tb in range(2):
                    for h in range(2):
                        last = img == IMGS_PER_CORE - 1 and tb == 1 and h == 1
                        if last:
                            do_group(img, h, tb, sub=(8, 4))
                            do_group(img, h, tb, sub=(12, 4))
                        else:
                            do_group(img, h, tb)
    nc.compile()
    return nc


def _get_nc():
    if not _NC_CACHE:
        _NC_CACHE.append(_build())
    return _NC_CACHE[0]


def _pack_weights(weights):
    # [cout, cin, kh, kw] -> Wt[v,kj,o,c] -> [cin, (h, nu, kj, coutl)]
    import ml_dtypes

    wt = np.einsum("vk,ockj->vjoc", _G, weights.astype(np.float64))
    # reorder to [c, h, nu, kj, ol]
    wt = wt.reshape(NNU, 3, 2, 128, CIN)  # v, kj, h, ol, c
    wtr = np.transpose(wt, (4, 2, 0, 1, 3))  # c, h, v, kj, ol
    return np.ascontiguousarray(wtr.reshape(CIN, 2 * NNU * 3 * 128)).astype(
        ml_dtypes.bfloat16
    )


def _pack_v(x):
    # x [n, cin, 64, 64] f32 -> V [n, cin, 6, 16, 64] bf16
    import ml_dtypes

    idx = _STARTS[:, None] + np.arange(6)[None, :]  # (16, 6)
    xg = x[:, :, idx, :]  # (n, c, 16, 6, 64)
    v = np.einsum("vj,nctjw->ncvtw", _BT.astype(np.float32), xg)
    return np.ascontiguousarray(v).astype(ml_dtypes.bfloat16)


def _make_in_maps(x, weights):
    xv = _pack_v(np.ascontiguousarray(x, dtype=np.float32))
    w_l = _pack_weights(np.ascontiguousarray(weights, dtype=np.float32))
    return [
        {"v": xv[IMGS_PER_CORE * c : IMGS_PER_CORE * (c + 1)], "w": w_l}
        for c in range(N_CORES)
    ]


def _ldw_opt_patch():
    """No-op (kept for harness compat)."""
    import contextlib

    return contextlib.nullcontext()


def _combine(m_all):
    # m_all: [16, 2, 128, 6, 16, 62] bf16 -> out [16, 256, 62, 62] f32
    m = np.asarray(m_all, dtype=np.float32)
    n = m.shape[0]
    m = m.reshape(n, 256, NNU, NT, OW)  # couts = h*128 + ol
    y = np.einsum("rv,novtu->notru", _AT.astype(np.float32), m)  # n,o,t,r,u
    out = np.empty((n, COUT, OH, OW), dtype=np.float32)
    for t in range(NT):
        out[:, :, _STARTS[t] : _STARTS[t] + 4, :] = y[:, :, t, :, :]
    return out


def kernel(x, weights):
    from concourse.bass_utils import run_bass_kernel_spmd

    nc = _get_nc()
    in_maps = _make_in_maps(x, weights)
    res = run_bass_kernel_spmd(nc, in_maps, core_ids=list(range(N_CORES)))
    m_all = np.concatenate([r["m"] for r in res.results], axis=0)
    return _combine(m_all)
